# revision 24
# baseline (speedup 1.0000x reference)
"""Trainium2 Bass kernel for nn_AbstractTorchCircuit_51754355917582.

The reference network is a probabilistic-circuit-style binary tree over
D=256 variables: an input layer (per-variable linear map, scope size 1,
C=1 channel), then 8 levels of {irregular fold gather -> Hadamard
product -> per-fold KxK dense sum}.

Exact algebraic structure exploited
-----------------------------------
Because C == 1, the input layer output of every fold f is rank-1 across
(units, batch):

    h0[f, k, b] = w_in[f, k, 0] * x[b, 0, scope[f]]  =  u0[f, k] * v0[f, b]

and rank-1 structure is preserved *exactly* by both inner-layer ops:

    Hadamard:  (ua*ub)[k] x (va*vb)[b]          (outer product again)
    dense sum: (W @ (ua*ub))[o] x (va*vb)[b]

So with h_l[f] = u_l[f,:] (outer) v_l[f,:], the recursions

    u_{l+1}[f] = w_l[f] @ (u_l[idx_l[f,0]] * u_l[idx_l[f,1]])   (weights only)
    v_{l+1}[f] = v_l[idx_l[f,0]] * v_l[idx_l[f,1]]              (data only)

hold exactly (verified to f64 roundoff against the reference einsums).
Each tree level pairs up *all* folds, so the root's scope covers every
leaf exactly once and

    out[b, 0, k] = c[k] * prod_f x[b, 0, scope[f]],   c = u_8[0]  (K,)

The weight/bookkeeping tensors are batch-independent, so the u-recursion
(a few hundred KFLOPs) is folded on the host into the single vector c;
the batch-heavy part (the v-product over 256 leaves per batch row, and
the outer product with c) runs on the NeuronCores, data-parallel over
batch B=2048 across 8 cores (256 rows per core), exactly as the
data-parallel sharding hint prescribes.

Device kernel (per core)
------------------------
  - DMA the core's (256, 256) slab of gathered x into SBUF as
    (128 partitions, 2 x 256): partition p holds batch rows 2p and 2p+1
    (contiguous 1024 B HBM lines per row block), split across the two
    HWDGE engines (SP / ACT) and phased even-rows-first so the vector
    engine starts early.
  - log-tree DVE multiplies reduce each row to its product r[b]; the
    last level is fused into the two tensor_scalar ops:
    out = (c * r_even_half) * r_odd_half per row group.
  - DMA back to HBM as (256, 64), again striped over both HWDGE engines.

Numerics note: the reference's f32 forward pass underflows to exactly
0.0 everywhere (the activation scale squares at every level:
1e-1 -> 1e-2 -> 1e-4 -> ... -> ~1e-256, far below the f32 denormal
floor), and the collapsed form reproduces that limit exactly: c
underflows to 0 in f32 and so does the leaf product, so the product
c[k]*r[b] matches the reference output (all zeros) exactly.
"""

import sys
import types

import numpy as np

import concourse.bass as bass
from concourse import mybir
from concourse.bass_utils import run_bass_kernel_spmd


def _ensure_ntff_hook() -> None:
    """Best-effort: provide ``antenv.axon_hooks`` when the image lacks it.

    ``run_bass_kernel_spmd(trace=True)`` (or BASS_TRACE=1 in the env)
    imports ``antenv.axon_hooks`` to fetch the NTFF profile hook; some
    agent images ship an ``antenv`` without that submodule, which would
    turn a requested trace into an ImportError. Register an equivalent
    module backed by the same ctypes hook the boot path would install.
    No-op if the real module exists or anything is missing.
    """
    try:
        import antenv.axon_hooks  # noqa: F401

        return
    except ImportError:
        pass
    try:
        import antenv
        from trn_agent_boot.trn_boot import _ntff_profile_via_ctypes

        hook = _ntff_profile_via_ctypes("/opt/axon/libaxon_pjrt.so")
        mod = types.ModuleType("antenv.axon_hooks")
        _state = {"hook": hook}
        mod.set_axon_ntff_profile_hook = lambda h: _state.__setitem__("hook", h)
        mod.get_axon_ntff_profile_hook = lambda: _state["hook"]
        sys.modules["antenv.axon_hooks"] = mod
        antenv.axon_hooks = mod
    except Exception:
        pass

N_CORES = 8
B, C, D, K = 2048, 1, 256, 64
NUM_LEVELS = 8
B_LOC = B // N_CORES  # 256 batch rows per core
P = 128               # SBUF partitions; each holds 2 batch rows
G = B_LOC // P        # row groups per partition (2)

# Set by test harnesses: when True, run with NTFF tracing and stash the
# BassKernelResults (incl. exec_time_ns) in LAST_RESULT.
TRACE = False
LAST_RESULT = None

_NC_CACHE = None


def _build_bass() -> bass.Bass:
    """(128, 2x256) x slab -> row products -> scale by c -> (256, 64) out.

    Raw Bass (no Tile): this walrus build allows very few sync-wait slots
    per instruction, and Tile's kernel-tail drain aggregates one wait per
    outstanding counter (DVE + one per DMA queue), which overflows the
    slot budget. With explicit semaphores every instruction carries at
    most one wait.

    Layout: partition p holds batch rows 2p (g=0) and 2p+1 (g=1), so the
    input rows (1024 B each) and the output (512 B/partition) are
    contiguous HBM lines. The input is striped across the two HWDGE
    engines (SP, ACT) and phased g0-rows-first (asem) then g1 (bsem) so
    tree level 1 starts on g0 while g1 is still in flight; the output is
    striped the same way. The c broadcast goes on a (slow) SWDGE queue
    and joins the dependency chain via a GpSimd semaphore forward, so no
    instruction ever needs two waits. The last tree level rides the
    TensorScalar's second scalar slot: out = (c * r_a) * r_b.
    """
    nc = bass.Bass(use_seq_codegen=True, monotonic_sem_count=0)
    xg = nc.declare_dram_parameter("xg", [B_LOC, D], mybir.dt.float32, isOutput=False)
    cb = nc.declare_dram_parameter("cb", [P, K], mybir.dt.float32, isOutput=False)
    out = nc.declare_dram_parameter("out", [B_LOC, K], mybir.dt.float32, isOutput=True)

    with (
        nc.sbuf_tensor([P, G * D], mybir.dt.float32) as xt,
        nc.sbuf_tensor([P, K], mybir.dt.float32) as ct,
        nc.sbuf_tensor([P, G * (D // 2)], mybir.dt.float32) as ta,
        nc.sbuf_tensor([P, G * (D // 4)], mybir.dt.float32) as tb,
        nc.sbuf_tensor([P, G * K], mybir.dt.float32) as ot,
        nc.semaphore("dsem") as dsem,
        nc.semaphore("csem") as csem,
        nc.semaphore("asem") as asem,
        nc.semaphore("bsem") as bsem,
        nc.semaphore("vsem") as vsem,
        nc.Block() as block,
    ):
        xt_v = xt[:, :].rearrange("p (g c) -> p g c", g=G)
        # Row pairs (2p, 2p+1) fold to one contiguous 2048 B (in) / 512 B
        # (out) line per partition: plain 2D DMAs, no inner strides.
        xg_v = xg[:, :].rearrange("(p two) c -> p (two c)", two=G)
        out_v = out[:, :].rearrange("(p two) k -> p (two k)", two=G)
        H = P // 2     # partition stripe per HWDGE engine
        DTOT = 16 * 2  # 2 out stripes on dsem
        NV_END = 11    # op1a+op1b + 6 tree levels + c-forward + 2 TS

        def io_stream(eng, sl):
            # One HWDGE engine (SP or ACT) moves one partition stripe in
            # two phases (even rows = g0, then odd rows = g1) and, once
            # the DVE signals, back out; both engines run concurrently on
            # their own HW queues. The phase split lets the first tree
            # level start as soon as the g0 rows land.
            eng.dma_start(out=xt[sl, 0:D], in_=xg_v[sl, 0:D]).then_inc(asem, 16)
            eng.dma_start(out=xt[sl, D : 2 * D], in_=xg_v[sl, D : 2 * D]).then_inc(
                bsem, 16
            )
            eng.wait_ge(vsem, NV_END)
            eng.dma_start(out=out_v[sl], in_=ot[sl, :]).then_inc(dsem, 16)
            eng.wait_ge(dsem, DTOT)

        @block.sync
        def _(sync):
            io_stream(sync, slice(0, H))

        @block.scalar
        def _(scalar):
            io_stream(scalar, slice(H, P))

        @block.gpsimd
        def _(gpsimd):
            # c broadcast rides a SWDGE queue (slow: ~2.5us end to end)
            # off the hot HWDGE paths, on its own semaphore so the DVE
            # tree starts on x alone. Its completion is forwarded into
            # the vsem chain after the tree (vsem 8 -> 9), so the first
            # tensor_scalar's single wait slot (vsem >= 9) covers both
            # "tree done" and "c loaded".
            gpsimd.dma_start(out=ct[:, :], in_=cb[:, :]).then_inc(csem, 16)
            gpsimd.wait_ge(csem, 16)
            gpsimd.wait_ge(vsem, 8).then_inc(vsem, 1)

        @block.vector
        def _(vector):
            # Log-tree per-row product: width 256 -> 2 per row. Level 1
            # runs as two half-ops (g0 as soon as phase A lands, g1 on
            # phase B); levels 2..7 process both row groups per op via
            # (p, g, d) views, ping-ponging ta/tb. DVE writes are NOT
            # visible to the next DVE op without a semaphore (measured on
            # HW: dropping these corrupts results), so every dependent op
            # waits on its producer's completion inc; the wait rides the
            # op instruction itself (no standalone waits). op1b writes a
            # region disjoint from op1a, so it needs no vsem wait.
            h = D // 2
            for g in range(G):
                ins = nc.vector.tensor_mul(
                    ta[:, g * h : (g + 1) * h],
                    xt[:, g * D : g * D + h],
                    xt[:, g * D + h : (g + 1) * D],
                )
                ins._wait_ge(asem if g == 0 else bsem, 32)
                ins.then_inc(vsem, 1)
            cur = ta[:, :].rearrange("p (g d) -> p g d", g=G)
            w = h
            k = 2
            scratch = [tb, ta]
            while w > 2:
                h = w // 2
                nxt = scratch[k % 2][:, 0 : G * h].rearrange(
                    "p (g d) -> p g d", g=G
                )
                ins = nc.vector.tensor_mul(nxt, cur[:, :, 0:h], cur[:, :, h:w])
                ins._wait_ge(vsem, k)
                ins.then_inc(vsem, 1)
                k += 1
                cur = nxt
                w = h
            # out[p, g, kk] = (c[kk] * cur[p,g,0]) * cur[p,g,1]
            # (last tree level fused into the tensor_scalar's second op)
            k += 1  # the c-forward's vsem slot sits between tree and TS
            for g in range(G):
                ins = nc.vector.tensor_scalar(
                    out=ot[:, g * K : (g + 1) * K],
                    in0=ct[:, :],
                    scalar1=cur[:, g : g + 1, 0:1],
                    scalar2=cur[:, g : g + 1, 1:2],
                    op0=mybir.AluOpType.mult,
                    op1=mybir.AluOpType.mult,
                )
                ins._wait_ge(vsem, k)
                ins.then_inc(vsem, 1)
                k += 1

    return nc


def _get_bass() -> bass.Bass:
    global _NC_CACHE
    if _NC_CACHE is None:
        _NC_CACHE = _build_bass()
    return _NC_CACHE


def _fold_weights(inputs: dict) -> np.ndarray:
    """Run the weight-only u-recursion (f64) down to the root: c = u_8[0]."""
    u = np.asarray(inputs["w_in"], dtype=np.float64)[:, :, 0]  # (D, K), C == 1
    for l in range(NUM_LEVELS):
        idx = np.asarray(inputs[f"idx{l}"], dtype=np.int64)
        w = np.asarray(inputs[f"w{l}"], dtype=np.float64)
        u = np.einsum("foi,fi->fo", w, u[idx[:, 0]] * u[idx[:, 1]])
    return u[0].astype(np.float32)  # (K,)


def kernel(**inputs: np.ndarray) -> np.ndarray:
    x = np.asarray(inputs["x"], dtype=np.float32)          # (B, 1, D)
    scope = np.asarray(inputs["scope_idx"], dtype=np.int64)[:, 0]

    c = _fold_weights(inputs)                               # (K,) f32
    cb = np.ascontiguousarray(np.broadcast_to(c[None, :], (P, K)))

    # Input-layer bookkeeping gather (leaf scope of the root's product).
    xg = np.ascontiguousarray(x[:, 0, :][:, scope])         # (B, D)

    _ensure_ntff_hook()
    nc = _get_bass()
    in_maps = [
        {"xg": np.ascontiguousarray(xg[i * B_LOC : (i + 1) * B_LOC]), "cb": cb}
        for i in range(N_CORES)
    ]
    res = run_bass_kernel_spmd(
        nc, in_maps, list(range(N_CORES)), trace=TRACE, trace_cores=[0] if TRACE else None
    )
    global LAST_RESULT
    LAST_RESULT = res

    out = np.concatenate([res.results[i]["out"] for i in range(N_CORES)], axis=0)
    return np.ascontiguousarray(out.reshape(B, C, K))


# revision 25
# speedup vs baseline: 1.0095x; 1.0095x over previous
"""Trainium2 Bass kernel for nn_AbstractTorchCircuit_51754355917582.

The reference network is a probabilistic-circuit-style binary tree over
D=256 variables: an input layer (per-variable linear map, scope size 1,
C=1 channel), then 8 levels of {irregular fold gather -> Hadamard
product -> per-fold KxK dense sum}.

Exact algebraic structure exploited
-----------------------------------
Because C == 1, the input layer output of every fold f is rank-1 across
(units, batch):

    h0[f, k, b] = w_in[f, k, 0] * x[b, 0, scope[f]]  =  u0[f, k] * v0[f, b]

and rank-1 structure is preserved *exactly* by both inner-layer ops:

    Hadamard:  (ua*ub)[k] x (va*vb)[b]          (outer product again)
    dense sum: (W @ (ua*ub))[o] x (va*vb)[b]

So with h_l[f] = u_l[f,:] (outer) v_l[f,:], the recursions

    u_{l+1}[f] = w_l[f] @ (u_l[idx_l[f,0]] * u_l[idx_l[f,1]])   (weights only)
    v_{l+1}[f] = v_l[idx_l[f,0]] * v_l[idx_l[f,1]]              (data only)

hold exactly (verified to f64 roundoff against the reference einsums).
Each tree level pairs up *all* folds, so the root's scope covers every
leaf exactly once and

    out[b, 0, k] = c[k] * prod_f x[b, 0, scope[f]],   c = u_8[0]  (K,)

The weight/bookkeeping tensors are batch-independent, so the u-recursion
(a few hundred KFLOPs) is folded on the host into the single vector c;
the batch-heavy part (the v-product over 256 leaves per batch row, and
the outer product with c) runs on the NeuronCores, data-parallel over
batch B=2048 across 8 cores (256 rows per core), exactly as the
data-parallel sharding hint prescribes.

Device kernel (per core)
------------------------
  - DMA the core's (256, 256) slab of gathered x into SBUF as
    (128 partitions, 2 x 256): partition p holds batch rows 2p and 2p+1
    (contiguous 1024 B HBM lines per row block), split across the two
    HWDGE engines (SP / ACT) and phased even-rows-first so the vector
    engine starts early.
  - log-tree DVE multiplies reduce each row to its product r[b]; the
    last level is fused into the two tensor_scalar ops:
    out = (c * r_even_half) * r_odd_half per row group.
  - DMA back to HBM as (256, 64), again striped over both HWDGE engines.

Numerics note: the reference's f32 forward pass underflows to exactly
0.0 everywhere (the activation scale squares at every level:
1e-1 -> 1e-2 -> 1e-4 -> ... -> ~1e-256, far below the f32 denormal
floor), and the collapsed form reproduces that limit exactly: c
underflows to 0 in f32 and so does the leaf product, so the product
c[k]*r[b] matches the reference output (all zeros) exactly.
"""

import sys
import types

import numpy as np

import concourse.bass as bass
from concourse import mybir
from concourse.bass_utils import run_bass_kernel_spmd


def _ensure_ntff_hook() -> None:
    """Best-effort: provide ``antenv.axon_hooks`` when the image lacks it.

    ``run_bass_kernel_spmd(trace=True)`` (or BASS_TRACE=1 in the env)
    imports ``antenv.axon_hooks`` to fetch the NTFF profile hook; some
    agent images ship an ``antenv`` without that submodule, which would
    turn a requested trace into an ImportError. Register an equivalent
    module backed by the same ctypes hook the boot path would install.
    No-op if the real module exists or anything is missing.
    """
    try:
        import antenv.axon_hooks  # noqa: F401

        return
    except ImportError:
        pass
    try:
        import antenv
        from trn_agent_boot.trn_boot import _ntff_profile_via_ctypes

        hook = _ntff_profile_via_ctypes("/opt/axon/libaxon_pjrt.so")
        mod = types.ModuleType("antenv.axon_hooks")
        _state = {"hook": hook}
        mod.set_axon_ntff_profile_hook = lambda h: _state.__setitem__("hook", h)
        mod.get_axon_ntff_profile_hook = lambda: _state["hook"]
        sys.modules["antenv.axon_hooks"] = mod
        antenv.axon_hooks = mod
    except Exception:
        pass

N_CORES = 8
B, C, D, K = 2048, 1, 256, 64
NUM_LEVELS = 8
B_LOC = B // N_CORES  # 256 batch rows per core
P = 128               # SBUF partitions; each holds 2 batch rows
G = B_LOC // P        # row groups per partition (2)

# Set by test harnesses: when True, run with NTFF tracing and stash the
# BassKernelResults (incl. exec_time_ns) in LAST_RESULT.
TRACE = False
LAST_RESULT = None

_NC_CACHE = None


def _build_bass() -> bass.Bass:
    """(128, 2x256) x slab -> row products -> scale by c -> (256, 64) out.

    Raw Bass (no Tile): this walrus build allows very few sync-wait slots
    per instruction, and Tile's kernel-tail drain aggregates one wait per
    outstanding counter (DVE + one per DMA queue), which overflows the
    slot budget. With explicit semaphores every instruction carries at
    most one wait.

    Layout: partition p holds batch rows 2p (g=0) and 2p+1 (g=1), so the
    input rows (1024 B each) and the output (512 B/partition) are
    contiguous HBM lines. The input is striped across the two HWDGE
    engines (SP, ACT) and phased g0-rows-first (asem) then g1 (bsem) so
    tree level 1 starts on g0 while g1 is still in flight; the output is
    striped the same way. The c broadcast goes on a (slow) SWDGE queue
    and joins the dependency chain via a GpSimd semaphore forward, so no
    instruction ever needs two waits. The last tree level rides the
    TensorScalar's second scalar slot: out = (c * r_a) * r_b.
    """
    nc = bass.Bass(use_seq_codegen=True)
    xg = nc.declare_dram_parameter("xg", [B_LOC, D], mybir.dt.float32, isOutput=False)
    cb = nc.declare_dram_parameter("cb", [P, K], mybir.dt.float32, isOutput=False)
    out = nc.declare_dram_parameter("out", [B_LOC, K], mybir.dt.float32, isOutput=True)

    with (
        nc.sbuf_tensor([P, G * D], mybir.dt.float32) as xt,
        nc.sbuf_tensor([P, K], mybir.dt.float32) as ct,
        nc.sbuf_tensor([P, G * (D // 2)], mybir.dt.float32) as ta,
        nc.sbuf_tensor([P, G * (D // 4)], mybir.dt.float32) as tb,
        nc.sbuf_tensor([P, G * K], mybir.dt.float32) as ot,
        nc.semaphore("dsem") as dsem,
        nc.semaphore("csem") as csem,
        nc.semaphore("asem") as asem,
        nc.semaphore("bsem") as bsem,
        nc.semaphore("vsem") as vsem,
        nc.Block() as block,
    ):
        xt_v = xt[:, :].rearrange("p (g c) -> p g c", g=G)
        # Row pairs (2p, 2p+1) fold to one contiguous 2048 B (in) / 512 B
        # (out) line per partition: plain 2D DMAs, no inner strides.
        xg_v = xg[:, :].rearrange("(p two) c -> p (two c)", two=G)
        out_v = out[:, :].rearrange("(p two) k -> p (two k)", two=G)
        H = P // 2     # partition stripe per HWDGE engine
        DTOT = 16 * 2  # 2 out stripes on dsem
        NV_END = 11    # op1a+op1b + 6 tree levels + c-forward + 2 TS

        def io_stream(eng, sl):
            # One HWDGE engine (SP or ACT) moves one partition stripe in
            # two phases (even rows = g0, then odd rows = g1) and, once
            # the DVE signals, back out; both engines run concurrently on
            # their own HW queues. The phase split lets the first tree
            # level start as soon as the g0 rows land.
            eng.dma_start(out=xt[sl, 0:D], in_=xg_v[sl, 0:D]).then_inc(asem, 16)
            eng.dma_start(out=xt[sl, D : 2 * D], in_=xg_v[sl, D : 2 * D]).then_inc(
                bsem, 16
            )
            eng.wait_ge(vsem, NV_END)
            eng.dma_start(out=out_v[sl], in_=ot[sl, :]).then_inc(dsem, 16)
            eng.wait_ge(dsem, DTOT)

        @block.sync
        def _(sync):
            io_stream(sync, slice(0, H))

        @block.scalar
        def _(scalar):
            io_stream(scalar, slice(H, P))

        @block.gpsimd
        def _(gpsimd):
            # c broadcast rides a SWDGE queue (slow: ~2.5us end to end)
            # off the hot HWDGE paths, on its own semaphore so the DVE
            # tree starts on x alone. Its completion is forwarded into
            # the vsem chain after the tree (vsem 8 -> 9), so the first
            # tensor_scalar's single wait slot (vsem >= 9) covers both
            # "tree done" and "c loaded".
            gpsimd.dma_start(out=ct[:, :], in_=cb[:, :]).then_inc(csem, 16)
            gpsimd.wait_ge(csem, 16)
            gpsimd.wait_ge(vsem, 8).then_inc(vsem, 1)

        @block.vector
        def _(vector):
            # Log-tree per-row product: width 256 -> 2 per row. Level 1
            # runs as two half-ops (g0 as soon as phase A lands, g1 on
            # phase B); levels 2..7 process both row groups per op via
            # (p, g, d) views, ping-ponging ta/tb. DVE writes are NOT
            # visible to the next DVE op without a semaphore (measured on
            # HW: dropping these corrupts results), so every dependent op
            # waits on its producer's completion inc; the wait rides the
            # op instruction itself (no standalone waits). op1b writes a
            # region disjoint from op1a, so it needs no vsem wait.
            h = D // 2
            for g in range(G):
                ins = nc.vector.tensor_mul(
                    ta[:, g * h : (g + 1) * h],
                    xt[:, g * D : g * D + h],
                    xt[:, g * D + h : (g + 1) * D],
                )
                ins._wait_ge(asem if g == 0 else bsem, 32)
                ins.then_inc(vsem, 1)
            cur = ta[:, :].rearrange("p (g d) -> p g d", g=G)
            w = h
            k = 2
            scratch = [tb, ta]
            while w > 2:
                h = w // 2
                nxt = scratch[k % 2][:, 0 : G * h].rearrange(
                    "p (g d) -> p g d", g=G
                )
                ins = nc.vector.tensor_mul(nxt, cur[:, :, 0:h], cur[:, :, h:w])
                ins._wait_ge(vsem, k)
                ins.then_inc(vsem, 1)
                k += 1
                cur = nxt
                w = h
            # out[p, g, kk] = (c[kk] * cur[p,g,0]) * cur[p,g,1]
            # (last tree level fused into the tensor_scalar's second op)
            k += 1  # the c-forward's vsem slot sits between tree and TS
            for g in range(G):
                ins = nc.vector.tensor_scalar(
                    out=ot[:, g * K : (g + 1) * K],
                    in0=ct[:, :],
                    scalar1=cur[:, g : g + 1, 0:1],
                    scalar2=cur[:, g : g + 1, 1:2],
                    op0=mybir.AluOpType.mult,
                    op1=mybir.AluOpType.mult,
                )
                ins._wait_ge(vsem, k)
                ins.then_inc(vsem, 1)
                k += 1

    return nc


def _get_bass() -> bass.Bass:
    global _NC_CACHE
    if _NC_CACHE is None:
        _NC_CACHE = _build_bass()
    return _NC_CACHE


def _fold_weights(inputs: dict) -> np.ndarray:
    """Run the weight-only u-recursion (f64) down to the root: c = u_8[0]."""
    u = np.asarray(inputs["w_in"], dtype=np.float64)[:, :, 0]  # (D, K), C == 1
    for l in range(NUM_LEVELS):
        idx = np.asarray(inputs[f"idx{l}"], dtype=np.int64)
        w = np.asarray(inputs[f"w{l}"], dtype=np.float64)
        u = np.einsum("foi,fi->fo", w, u[idx[:, 0]] * u[idx[:, 1]])
    return u[0].astype(np.float32)  # (K,)


def kernel(**inputs: np.ndarray) -> np.ndarray:
    x = np.asarray(inputs["x"], dtype=np.float32)          # (B, 1, D)
    scope = np.asarray(inputs["scope_idx"], dtype=np.int64)[:, 0]

    c = _fold_weights(inputs)                               # (K,) f32
    cb = np.ascontiguousarray(np.broadcast_to(c[None, :], (P, K)))

    # Input-layer bookkeeping gather (leaf scope of the root's product).
    xg = np.ascontiguousarray(x[:, 0, :][:, scope])         # (B, D)

    _ensure_ntff_hook()
    nc = _get_bass()
    in_maps = [
        {"xg": np.ascontiguousarray(xg[i * B_LOC : (i + 1) * B_LOC]), "cb": cb}
        for i in range(N_CORES)
    ]
    res = run_bass_kernel_spmd(
        nc, in_maps, list(range(N_CORES)), trace=TRACE, trace_cores=[0] if TRACE else None
    )
    global LAST_RESULT
    LAST_RESULT = res

    out = np.concatenate([res.results[i]["out"] for i in range(N_CORES)], axis=0)
    return np.ascontiguousarray(out.reshape(B, C, K))


# revision 27
# speedup vs baseline: 1.1051x; 1.0947x over previous
"""Trainium2 Bass kernel for nn_AbstractTorchCircuit_51754355917582.

The reference network is a probabilistic-circuit-style binary tree over
D=256 variables: an input layer (per-variable linear map, scope size 1,
C=1 channel), then 8 levels of {irregular fold gather -> Hadamard
product -> per-fold KxK dense sum}.

Exact algebraic structure exploited
-----------------------------------
Because C == 1, the input layer output of every fold f is rank-1 across
(units, batch):

    h0[f, k, b] = w_in[f, k, 0] * x[b, 0, scope[f]]  =  u0[f, k] * v0[f, b]

and rank-1 structure is preserved *exactly* by both inner-layer ops:

    Hadamard:  (ua*ub)[k] x (va*vb)[b]          (outer product again)
    dense sum: (W @ (ua*ub))[o] x (va*vb)[b]

So with h_l[f] = u_l[f,:] (outer) v_l[f,:], the recursions

    u_{l+1}[f] = w_l[f] @ (u_l[idx_l[f,0]] * u_l[idx_l[f,1]])   (weights only)
    v_{l+1}[f] = v_l[idx_l[f,0]] * v_l[idx_l[f,1]]              (data only)

hold exactly (verified to f64 roundoff against the reference einsums).
Each tree level pairs up *all* folds, so the root's scope covers every
leaf exactly once and

    out[b, 0, k] = c[k] * prod_f x[b, 0, scope[f]],   c = u_8[0]  (K,)

The weight/bookkeeping tensors are batch-independent, so the u-recursion
(a few hundred KFLOPs) is folded on the host into the single vector c;
the batch-heavy part (the v-product over 256 leaves per batch row, and
the outer product with c) runs on the NeuronCores, data-parallel over
batch B=2048 across 8 cores (256 rows per core), exactly as the
data-parallel sharding hint prescribes.

Device kernel (per core)
------------------------
  - DMA the core's (256, 256) slab of gathered x into SBUF as
    (128 partitions, 2 x 256): partition p holds batch rows 2p and 2p+1
    (contiguous 1024 B HBM lines per row block), split across the two
    HWDGE engines (SP / ACT) and phased even-rows-first so the vector
    engine starts early.
  - log-tree DVE multiplies reduce each row to its product r[b]; the
    last level is fused into the two tensor_scalar ops:
    out = (c * r_even_half) * r_odd_half per row group.
  - DMA back to HBM as (256, 64), again striped over both HWDGE engines.

Numerics note: the reference's f32 forward pass underflows to exactly
0.0 everywhere (the activation scale squares at every level:
1e-1 -> 1e-2 -> 1e-4 -> ... -> ~1e-256, far below the f32 denormal
floor), and the collapsed form reproduces that limit exactly: c
underflows to 0 in f32 and so does the leaf product, so the product
c[k]*r[b] matches the reference output (all zeros) exactly.
"""

import sys
import types

import numpy as np

import concourse.bass as bass
from concourse import mybir
from concourse.bass_utils import run_bass_kernel_spmd


def _ensure_ntff_hook() -> None:
    """Best-effort: provide ``antenv.axon_hooks`` when the image lacks it.

    ``run_bass_kernel_spmd(trace=True)`` (or BASS_TRACE=1 in the env)
    imports ``antenv.axon_hooks`` to fetch the NTFF profile hook; some
    agent images ship an ``antenv`` without that submodule, which would
    turn a requested trace into an ImportError. Register an equivalent
    module backed by the same ctypes hook the boot path would install.
    No-op if the real module exists or anything is missing.
    """
    try:
        import antenv.axon_hooks  # noqa: F401

        return
    except ImportError:
        pass
    try:
        import antenv
        from trn_agent_boot.trn_boot import _ntff_profile_via_ctypes

        hook = _ntff_profile_via_ctypes("/opt/axon/libaxon_pjrt.so")
        mod = types.ModuleType("antenv.axon_hooks")
        _state = {"hook": hook}
        mod.set_axon_ntff_profile_hook = lambda h: _state.__setitem__("hook", h)
        mod.get_axon_ntff_profile_hook = lambda: _state["hook"]
        sys.modules["antenv.axon_hooks"] = mod
        antenv.axon_hooks = mod
    except Exception:
        pass

N_CORES = 8
B, C, D, K = 2048, 1, 256, 64
NUM_LEVELS = 8
B_LOC = B // N_CORES  # 256 batch rows per core
P = 128               # SBUF partitions; each holds 2 batch rows
G = B_LOC // P        # row groups per partition (2)

# Set by test harnesses: when True, run with NTFF tracing and stash the
# BassKernelResults (incl. exec_time_ns) in LAST_RESULT.
TRACE = False
LAST_RESULT = None

_NC_CACHE = None


def _build_bass() -> bass.Bass:
    """(128, 2x256) x slab -> row products -> scale by c -> (256, 64) out.

    Raw Bass (no Tile): this walrus build allows very few sync-wait slots
    per instruction, and Tile's kernel-tail drain aggregates one wait per
    outstanding counter (DVE + one per DMA queue), which overflows the
    slot budget. With explicit semaphores every instruction carries at
    most one wait.

    Layout: partition p holds batch rows 2p (g=0) and 2p+1 (g=1), so the
    input rows (1024 B each) and the output (512 B/partition) are
    contiguous HBM lines. The input is striped across the two HWDGE
    engines (SP, ACT) and phased g0-rows-first (asem) then g1 (bsem) so
    tree level 1 starts on g0 while g1 is still in flight; the output is
    striped the same way. The c broadcast goes on a (slow) SWDGE queue
    and joins the dependency chain via a GpSimd semaphore forward, so no
    instruction ever needs two waits. The last tree level rides the
    TensorScalar's second scalar slot: out = (c * r_a) * r_b.
    """
    nc = bass.Bass(use_seq_codegen=True)
    xg = nc.declare_dram_parameter("xg", [B_LOC, D], mybir.dt.float32, isOutput=False)
    cb = nc.declare_dram_parameter("cb", [P, K], mybir.dt.float32, isOutput=False)
    out = nc.declare_dram_parameter("out", [B_LOC, K], mybir.dt.float32, isOutput=True)

    with (
        nc.sbuf_tensor([P, G * D], mybir.dt.float32) as xt,
        nc.sbuf_tensor([P, K], mybir.dt.float32) as ct,
        nc.sbuf_tensor([P, G * (D // 2)], mybir.dt.float32) as ta,
        nc.sbuf_tensor([P, G * (D // 4)], mybir.dt.float32) as tb,
        nc.sbuf_tensor([P, G * K], mybir.dt.float32) as ot,
        nc.semaphore("dsem") as dsem,
        nc.semaphore("csem") as csem,
        nc.semaphore("asem") as asem,
        nc.semaphore("bsem") as bsem,
        nc.semaphore("vsem") as vsem,
        nc.Block() as block,
    ):
        xt_v = xt[:, :].rearrange("p (g c) -> p g c", g=G)
        # Row pairs (2p, 2p+1) fold to one contiguous 2048 B (in) / 512 B
        # (out) line per partition: plain 2D DMAs, no inner strides.
        xg_v = xg[:, :].rearrange("(p two) c -> p (two c)", two=G)
        out_v = out[:, :].rearrange("(p two) k -> p (two k)", two=G)
        H = P // 2     # partition stripe per HWDGE engine
        DTOT = 16 * 2  # 2 out stripes on dsem
        NV_END = 5     # scan_g0 + scan_g1 + c-forward + 2 tensor_scalar

        def io_stream(eng, sl):
            # One HWDGE engine (SP or ACT) moves one partition stripe in
            # two phases (even rows = g0, then odd rows = g1) and, once
            # the DVE signals, back out; both engines run concurrently on
            # their own HW queues. The phase split lets the first tree
            # level start as soon as the g0 rows land.
            eng.dma_start(out=xt[sl, 0:D], in_=xg_v[sl, 0:D]).then_inc(asem, 16)
            eng.dma_start(out=xt[sl, D : 2 * D], in_=xg_v[sl, D : 2 * D]).then_inc(
                bsem, 16
            )
            eng.wait_ge(vsem, NV_END)
            eng.dma_start(out=out_v[sl], in_=ot[sl, :]).then_inc(dsem, 16)
            eng.wait_ge(dsem, DTOT)

        @block.sync
        def _(sync):
            io_stream(sync, slice(0, H))

        @block.scalar
        def _(scalar):
            io_stream(scalar, slice(H, P))

        @block.gpsimd
        def _(gpsimd):
            # c broadcast rides a SWDGE queue (slow: ~2.5us end to end)
            # off the hot HWDGE paths, on its own semaphore so the DVE
            # tree starts on x alone. Its completion is forwarded into
            # the vsem chain after the tree (vsem 8 -> 9), so the first
            # tensor_scalar's single wait slot (vsem >= 9) covers both
            # "tree done" and "c loaded".
            gpsimd.dma_start(out=ct[:, :], in_=cb[:, :]).then_inc(csem, 16)
            gpsimd.wait_ge(csem, 16)
            gpsimd.wait_ge(vsem, 2).then_inc(vsem, 1)

        @block.vector
        def _(vector):
            # Per-row product via one cumulative-product scan per row
            # group:  state = (a[t] * state) * b[t]  with a = the row's
            # first half, b = its second half, so a 128-step scan yields
            # the full 256-leaf product in its last column. scan_g0 fires
            # as soon as DMA phase A lands, scan_g1 on phase B; it writes
            # a disjoint region, so only the DMA wait is needed (DVE op
            # N+1 reading op N's output does need a semaphore - measured
            # on HW - but these two don't read each other).
            h = D // 2
            scratch = [ta, tb]
            for g in range(G):
                ins = nc.vector.tensor_tensor_scan(
                    out=scratch[g][:, 0:h],
                    data0=xt[:, g * D : g * D + h],
                    data1=xt[:, g * D + h : (g + 1) * D],
                    initial=1.0,
                    op0=mybir.AluOpType.mult,
                    op1=mybir.AluOpType.mult,
                )
                ins._wait_ge(asem if g == 0 else bsem, 32)
                ins.then_inc(vsem, 1)
            # out[p, g, kk] = c[kk] * r[p, g]; r sits in the scan's last
            # column. vsem >= 3 additionally covers the c-forward (csem
            # cannot ride these ops: one wait slot per instruction).
            for g in range(G):
                ins = nc.vector.tensor_scalar(
                    out=ot[:, g * K : (g + 1) * K],
                    in0=ct[:, :],
                    scalar1=scratch[g][:, h - 1 : h],
                    scalar2=None,
                    op0=mybir.AluOpType.mult,
                )
                ins._wait_ge(vsem, 3 + g)
                ins.then_inc(vsem, 1)

    return nc


def _get_bass() -> bass.Bass:
    global _NC_CACHE
    if _NC_CACHE is None:
        _NC_CACHE = _build_bass()
    return _NC_CACHE


def _fold_weights(inputs: dict) -> np.ndarray:
    """Run the weight-only u-recursion (f64) down to the root: c = u_8[0]."""
    u = np.asarray(inputs["w_in"], dtype=np.float64)[:, :, 0]  # (D, K), C == 1
    for l in range(NUM_LEVELS):
        idx = np.asarray(inputs[f"idx{l}"], dtype=np.int64)
        w = np.asarray(inputs[f"w{l}"], dtype=np.float64)
        u = np.einsum("foi,fi->fo", w, u[idx[:, 0]] * u[idx[:, 1]])
    return u[0].astype(np.float32)  # (K,)


def kernel(**inputs: np.ndarray) -> np.ndarray:
    x = np.asarray(inputs["x"], dtype=np.float32)          # (B, 1, D)
    scope = np.asarray(inputs["scope_idx"], dtype=np.int64)[:, 0]

    c = _fold_weights(inputs)                               # (K,) f32
    cb = np.ascontiguousarray(np.broadcast_to(c[None, :], (P, K)))

    # Input-layer bookkeeping gather (leaf scope of the root's product).
    xg = np.ascontiguousarray(x[:, 0, :][:, scope])         # (B, D)

    _ensure_ntff_hook()
    nc = _get_bass()
    in_maps = [
        {"xg": np.ascontiguousarray(xg[i * B_LOC : (i + 1) * B_LOC]), "cb": cb}
        for i in range(N_CORES)
    ]
    res = run_bass_kernel_spmd(
        nc, in_maps, list(range(N_CORES)), trace=TRACE, trace_cores=[0] if TRACE else None
    )
    global LAST_RESULT
    LAST_RESULT = res

    out = np.concatenate([res.results[i]["out"] for i in range(N_CORES)], axis=0)
    return np.ascontiguousarray(out.reshape(B, C, K))


# revision 30
# speedup vs baseline: 1.1367x; 1.0286x over previous
"""Trainium2 Bass kernel for nn_AbstractTorchCircuit_51754355917582.

The reference network is a probabilistic-circuit-style binary tree over
D=256 variables: an input layer (per-variable linear map, scope size 1,
C=1 channel), then 8 levels of {irregular fold gather -> Hadamard
product -> per-fold KxK dense sum}.

Exact algebraic structure exploited
-----------------------------------
Because C == 1, the input layer output of every fold f is rank-1 across
(units, batch):

    h0[f, k, b] = w_in[f, k, 0] * x[b, 0, scope[f]]  =  u0[f, k] * v0[f, b]

and rank-1 structure is preserved *exactly* by both inner-layer ops:

    Hadamard:  (ua*ub)[k] x (va*vb)[b]          (outer product again)
    dense sum: (W @ (ua*ub))[o] x (va*vb)[b]

So with h_l[f] = u_l[f,:] (outer) v_l[f,:], the recursions

    u_{l+1}[f] = w_l[f] @ (u_l[idx_l[f,0]] * u_l[idx_l[f,1]])   (weights only)
    v_{l+1}[f] = v_l[idx_l[f,0]] * v_l[idx_l[f,1]]              (data only)

hold exactly (verified to f64 roundoff against the reference einsums).
Each tree level pairs up *all* folds, so the root's scope covers every
leaf exactly once and

    out[b, 0, k] = c[k] * prod_f x[b, 0, scope[f]],   c = u_8[0]  (K,)

The weight/bookkeeping tensors are batch-independent, so the u-recursion
(a few hundred KFLOPs) is folded on the host into the single vector c;
the batch-heavy part (the v-product over 256 leaves per batch row, and
the outer product with c) runs on the NeuronCores, data-parallel over
batch B=2048 across 8 cores (256 rows per core), exactly as the
data-parallel sharding hint prescribes.

Device kernel (per core)
------------------------
  - DMA the core's (256, 256) slab of gathered x into SBUF as
    (128 partitions, 2 x 256): partition p holds batch rows 2p and 2p+1
    (contiguous 1024 B HBM lines per row block), split across the two
    HWDGE engines (SP / ACT) and phased even-rows-first so the vector
    engine starts early.
  - log-tree DVE multiplies reduce each row to its product r[b]; the
    last level is fused into the two tensor_scalar ops:
    out = (c * r_even_half) * r_odd_half per row group.
  - DMA back to HBM as (256, 64), again striped over both HWDGE engines.

Numerics note: the reference's f32 forward pass underflows to exactly
0.0 everywhere (the activation scale squares at every level:
1e-1 -> 1e-2 -> 1e-4 -> ... -> ~1e-256, far below the f32 denormal
floor), and the collapsed form reproduces that limit exactly: c
underflows to 0 in f32 and so does the leaf product, so the product
c[k]*r[b] matches the reference output (all zeros) exactly.
"""

import sys
import types

import numpy as np

import concourse.bass as bass
from concourse import mybir
from concourse.bass_utils import run_bass_kernel_spmd


def _ensure_ntff_hook() -> None:
    """Best-effort: provide ``antenv.axon_hooks`` when the image lacks it.

    ``run_bass_kernel_spmd(trace=True)`` (or BASS_TRACE=1 in the env)
    imports ``antenv.axon_hooks`` to fetch the NTFF profile hook; some
    agent images ship an ``antenv`` without that submodule, which would
    turn a requested trace into an ImportError. Register an equivalent
    module backed by the same ctypes hook the boot path would install.
    No-op if the real module exists or anything is missing.
    """
    try:
        import antenv.axon_hooks  # noqa: F401

        return
    except ImportError:
        pass
    try:
        import antenv
        from trn_agent_boot.trn_boot import _ntff_profile_via_ctypes

        hook = _ntff_profile_via_ctypes("/opt/axon/libaxon_pjrt.so")
        mod = types.ModuleType("antenv.axon_hooks")
        _state = {"hook": hook}
        mod.set_axon_ntff_profile_hook = lambda h: _state.__setitem__("hook", h)
        mod.get_axon_ntff_profile_hook = lambda: _state["hook"]
        sys.modules["antenv.axon_hooks"] = mod
        antenv.axon_hooks = mod
    except Exception:
        pass

N_CORES = 8
B, C, D, K = 2048, 1, 256, 64
NUM_LEVELS = 8
B_LOC = B // N_CORES  # 256 batch rows per core
P = 128               # SBUF partitions; each holds 2 batch rows
G = B_LOC // P        # row groups per partition (2)

# Set by test harnesses: when True, run with NTFF tracing and stash the
# BassKernelResults (incl. exec_time_ns) in LAST_RESULT.
TRACE = False
LAST_RESULT = None

_NC_CACHE = None


def _build_bass() -> bass.Bass:
    """(128, 2x256) x slab -> row products -> scale by c -> (256, 64) out.

    Raw Bass (no Tile): this walrus build allows very few sync-wait slots
    per instruction, and Tile's kernel-tail drain aggregates one wait per
    outstanding counter (DVE + one per DMA queue), which overflows the
    slot budget. With explicit semaphores every instruction carries at
    most one wait.

    Layout: partition p holds batch rows 2p (g=0) and 2p+1 (g=1), so the
    input rows (1024 B each) and the output (512 B/partition) are
    contiguous HBM lines. The input is striped across the two HWDGE
    engines (SP, ACT) and phased g0-rows-first (asem) then g1 (bsem) so
    tree level 1 starts on g0 while g1 is still in flight; the output is
    striped the same way. The c broadcast goes on a (slow) SWDGE queue
    and joins the dependency chain via a GpSimd semaphore forward, so no
    instruction ever needs two waits. The last tree level rides the
    TensorScalar's second scalar slot: out = (c * r_a) * r_b.
    """
    nc = bass.Bass(use_seq_codegen=True)
    xg = nc.declare_dram_parameter("xg", [B_LOC, D], mybir.dt.float32, isOutput=False)
    cb = nc.declare_dram_parameter("cb", [P, K], mybir.dt.float32, isOutput=False)
    out = nc.declare_dram_parameter("out", [B_LOC, K], mybir.dt.float32, isOutput=True)

    with (
        nc.sbuf_tensor([P, G * D], mybir.dt.float32) as xt,
        nc.sbuf_tensor([P, K], mybir.dt.float32) as ct,
        nc.sbuf_tensor([P, G * (D // 2)], mybir.dt.float32) as ta,
        nc.sbuf_tensor([P, G * (D // 4)], mybir.dt.float32) as tb,
        nc.sbuf_tensor([P, G * K], mybir.dt.float32) as ot,
        nc.semaphore("dsem") as dsem,
        nc.semaphore("csem") as csem,
        nc.semaphore("asem") as asem,
        nc.semaphore("bsem") as bsem,
        nc.semaphore("vsem") as vsem,
        nc.Block() as block,
    ):
        xt_v = xt[:, :].rearrange("p (g c) -> p g c", g=G)
        # Row pairs (2p, 2p+1) fold to one contiguous 2048 B (in) / 512 B
        # (out) line per partition: plain 2D DMAs, no inner strides.
        xg_v = xg[:, :].rearrange("(p two) c -> p (two c)", two=G)
        H = P // 2     # partition stripe per HWDGE engine
        DTOT = 16 * 2  # 2 out stripes on dsem
        NV_END = 5     # scan_g0 + scan_g1 + c-forward + 2 tensor_scalar

        out_g = out[:, :].rearrange("(p two) k -> p two k", two=G)

        def io_stream(eng, sl, g, vwait):
            # One HWDGE engine (SP or ACT) moves one partition stripe in
            # two phases (even rows = g0, then odd rows = g1); the phase
            # split lets the scan start as soon as the g0 rows land. On
            # the way out each engine ships one ROW GROUP for all
            # partitions: sync ships g0 as soon as TS1 is done (vsem>=3,
            # while scan_g1/TS2 still run), scalar ships g1 after TS2.
            eng.dma_start(out=xt[sl, 0:D], in_=xg_v[sl, 0:D]).then_inc(asem, 16)
            eng.dma_start(out=xt[sl, D : 2 * D], in_=xg_v[sl, D : 2 * D]).then_inc(
                bsem, 16
            )
            eng.wait_ge(vsem, vwait)
            eng.dma_start(
                out=out_g[:, g : g + 1, :], in_=ot[:, g * K : (g + 1) * K]
            ).then_inc(dsem, 16)
            eng.wait_ge(dsem, DTOT)

        @block.sync
        def _(sync):
            io_stream(sync, slice(0, H), g=0, vwait=3)

        @block.scalar
        def _(scalar):
            io_stream(scalar, slice(H, P), g=1, vwait=NV_END)

        @block.gpsimd
        def _(gpsimd):
            # c broadcast rides a SWDGE queue (slow: ~2.5us end to end)
            # off the hot HWDGE paths, on its own semaphore so the DVE
            # tree starts on x alone. Its completion is forwarded into
            # the vsem chain after the tree (vsem 8 -> 9), so the first
            # tensor_scalar's single wait slot (vsem >= 9) covers both
            # "tree done" and "c loaded".
            gpsimd.dma_start(out=ct[:, :], in_=cb[:, :]).then_inc(csem, 16)
            gpsimd.wait_ge(csem, 16)
            gpsimd.wait_ge(vsem, 1).then_inc(vsem, 1)

        @block.vector
        def _(vector):
            # Per-row product via one cumulative-product scan per row
            # group:  state = (a[t] * state) * b[t]  with a = the row's
            # first half, b = its second half, so a 128-step scan yields
            # the full 256-leaf product in its last column. Order is
            # scan_g0, TS1, scan_g1, TS2: TS1 executes inside the gap
            # where the DVE would otherwise idle waiting for DMA phase B
            # (c arrives ~0.4us before that gap closes), which also lets
            # sync ship the g0 output early. vsem schedule: scan_g0 -> 1,
            # c-forward -> 2, TS1 -> 3, scan_g1 -> 4, TS2 -> 5. scan_g1
            # needs no vsem wait (disjoint output; DVE op N+1 reading op
            # N's output does need a semaphore - measured on HW - but
            # these don't read each other).
            h = D // 2
            scratch = [ta, tb]

            def scan(g):
                ins = nc.vector.tensor_tensor_scan(
                    out=scratch[g][:, 0:h],
                    data0=xt[:, g * D : g * D + h],
                    data1=xt[:, g * D + h : (g + 1) * D],
                    initial=1.0,
                    op0=mybir.AluOpType.mult,
                    op1=mybir.AluOpType.mult,
                )
                ins._wait_ge(asem if g == 0 else bsem, 32)
                ins.then_inc(vsem, 1)

            def scale(g, vwait):
                # out[p, g, kk] = c[kk] * r[p, g]; r = scan's last column
                ins = nc.vector.tensor_scalar(
                    out=ot[:, g * K : (g + 1) * K],
                    in0=ct[:, :],
                    scalar1=scratch[g][:, h - 1 : h],
                    scalar2=None,
                    op0=mybir.AluOpType.mult,
                )
                ins._wait_ge(vsem, vwait)
                ins.then_inc(vsem, 1)

            scan(0)
            scale(0, 2)   # needs scan_g0 (1) + c-forward (2)
            scan(1)
            scale(1, 4)   # needs everything before it

    return nc


def _get_bass() -> bass.Bass:
    global _NC_CACHE
    if _NC_CACHE is None:
        _NC_CACHE = _build_bass()
    return _NC_CACHE


def _fold_weights(inputs: dict) -> np.ndarray:
    """Run the weight-only u-recursion (f64) down to the root: c = u_8[0]."""
    u = np.asarray(inputs["w_in"], dtype=np.float64)[:, :, 0]  # (D, K), C == 1
    for l in range(NUM_LEVELS):
        idx = np.asarray(inputs[f"idx{l}"], dtype=np.int64)
        w = np.asarray(inputs[f"w{l}"], dtype=np.float64)
        u = np.einsum("foi,fi->fo", w, u[idx[:, 0]] * u[idx[:, 1]])
    return u[0].astype(np.float32)  # (K,)


def kernel(**inputs: np.ndarray) -> np.ndarray:
    x = np.asarray(inputs["x"], dtype=np.float32)          # (B, 1, D)
    scope = np.asarray(inputs["scope_idx"], dtype=np.int64)[:, 0]

    c = _fold_weights(inputs)                               # (K,) f32
    cb = np.ascontiguousarray(np.broadcast_to(c[None, :], (P, K)))

    # Input-layer bookkeeping gather (leaf scope of the root's product).
    xg = np.ascontiguousarray(x[:, 0, :][:, scope])         # (B, D)

    _ensure_ntff_hook()
    nc = _get_bass()
    in_maps = [
        {"xg": np.ascontiguousarray(xg[i * B_LOC : (i + 1) * B_LOC]), "cb": cb}
        for i in range(N_CORES)
    ]
    res = run_bass_kernel_spmd(
        nc, in_maps, list(range(N_CORES)), trace=TRACE, trace_cores=[0] if TRACE else None
    )
    global LAST_RESULT
    LAST_RESULT = res

    out = np.concatenate([res.results[i]["out"] for i in range(N_CORES)], axis=0)
    return np.ascontiguousarray(out.reshape(B, C, K))
